# revision 60
# baseline (speedup 1.0000x reference)
"""Trainium2 Bass kernel for nn_Attention_50500225466997.

Computation (per batch): qkv = BN(conv1x1(x)); 4-head attention over L=1024
(DK=32, DH=64); out = attn + BN(dwconv3x3(v)); y = BN(conv1x1(out)).

Strategy:
  - Data-parallel over batch: 16 batches -> 8 NeuronCores, 2 per core.
  - All BN scales/permutations/SCALE folded into weights on the host.
  - Scores computed TRANSPOSED (S^T[l, m], l on partitions) so softmax
    needs no P-transposes: E = exp(S^T) unnormalized; Z rides FREE in the
    V-matmul by augmenting each head's vT lhsT with a 64-wide ones block
    ([vT_e|1] / [1|vT_o]) -> banks [O_e;Z_e] / [Z_o;O_o]; 1/Z via one
    exact DVE reciprocal, halves swapped by SBUF->SBUF DMA.
  - fp32r matmuls (fast fp32 streaming mode, ~tf32-accumulate-fp32).
  - Depthwise 3x3 via 9 diagonal-matrix matmuls accumulating in PSUM.
  - Cross-batch software pipelining: batch b+1's qkv/transposes emitted
    before batch b's depthwise/pointwise tail.
"""

import numpy as np

import concourse.bass as bass
import concourse.mybir as mybir
import concourse.tile as tile
from concourse import bacc
from concourse.bass_utils import run_bass_kernel_spmd

F32 = mybir.dt.float32
F32R = mybir.dt.float32r
AF = mybir.ActivationFunctionType
OP = mybir.AluOpType

B, CH, HH, WW = 16, 256, 32, 32
L = HH * WW                   # 1024
NH, DK, DH = 4, 32, 64
CQKV = CH + DK * NH * 2       # 512
SCALE = DK ** (-0.5)
NCORES = 8
BL = B // NCORES              # batches per core

# dtype config for precision experiments: flags switch a stage's matmul
# operands to full fp32 (4 cyc/row) instead of fp32r (1 cyc/row).
CFG = {"qkv32": False, "sc32": False, "o32": False, "pw32": False, "dw32": False, "o_bf16": True}




def build_bass():
    nc = bacc.Bacc("TRN2", target_bir_lowering=False, debug=False)
    BF16 = mybir.dt.bfloat16
    DT_XW = BF16
    DT_QK = BF16
    DT_VT = BF16 if CFG["o_bf16"] else (F32 if CFG["o32"] else F32R)
    DT_PW = BF16
    DT_DW = BF16

    x_d = nc.dram_tensor("x", [BL, CH, L], DT_XW, kind="ExternalInput")
    wqkvT_d = nc.dram_tensor("wqkvT", [128, 2, CQKV], DT_XW, kind="ExternalInput")
    bqkv_d = nc.dram_tensor("bqkv", [128, 4], F32, kind="ExternalInput")
    wpwT_d = nc.dram_tensor("wpwT", [128, 2, CH], DT_PW, kind="ExternalInput")
    bpw_d = nc.dram_tensor("bpw", [128, 2], F32, kind="ExternalInput")
    diag_d = nc.dram_tensor("diag", [128, 18, 128], DT_DW, kind="ExternalInput")
    ones_d = nc.dram_tensor("ones512", [128, 512], DT_VT, kind="ExternalInput")
    out_d = nc.dram_tensor("out", [BL, CH, L], F32, kind="ExternalOutput")

    with tile.TileContext(nc) as tc, nc.allow_low_precision(reason="fp32r"):
        with (
            tc.tile_pool(name="consts", bufs=1) as consts,
            tc.tile_pool(name="xin", bufs=4) as xin,
            tc.tile_pool(name="qkv", bufs=2) as qkvp,
            tc.tile_pool(name="vt", bufs=1) as vtp,
            tc.tile_pool(name="et", bufs=6) as etp,
            tc.tile_pool(name="o2", bufs=4) as o2p,
            tc.tile_pool(name="small", bufs=4) as smallp,
            tc.tile_pool(name="pad", bufs=4) as padp,
            tc.tile_pool(name="outp", bufs=4) as outp,
            tc.tile_pool(name="psc", bufs=2, space="PSUM") as psc,
            tc.tile_pool(name="pO", bufs=2, space="PSUM") as pOp,
            tc.tile_pool(name="pwork", bufs=2, space="PSUM") as pwork,
        ):
            # ---------------- constants (qkv path first) ----------------
            # DMA order tuned for startup latency: K/Q weights first, then
            # batch-0 x in half-tiles (first compute needs only the first
            # halves).  Startup DMAs are spread across issuing engines so
            # the per-DMA fixed overhead overlaps (separate HWDGE FIFOs).
            wqkvT = consts.tile([128, 2, CQKV], DT_XW)
            bqkv = consts.tile([128, 4], F32)
            nc.sync.dma_start(bqkv, bqkv_d.ap())
            nc.sync.dma_start(wqkvT[:, :, 0:256], wqkvT_d.ap()[:, :, 0:256])
            ones512 = consts.tile([128, 8, 64], DT_VT)
            wpwT = consts.tile([128, 2, CH], DT_PW)
            bpw = consts.tile([128, 2], F32)
            diag = consts.tile([128, 18, 128], DT_DW)

            # prefetch all batches' x (xin bufs cover BL*2 tiles)
            Xall = []
            for b in range(BL):
                Xb = []
                for ct in range(2):
                    xt = xin.tile([128, L], DT_XW, name=f"x_b{b}c{ct}", tag="x")
                    Xb.append(xt)
                if b == 0:
                    # quarter-granularity for the first half (first qkv
                    # chunks start sooner), halves for the rest
                    qeng = [nc.scalar, nc.gpsimd]
                    for q in range(2):
                        for ct in range(2):
                            qs = slice(256 * q, 256 * q + 256)
                            qeng[ct].dma_start(
                                Xb[ct][:, qs],
                                x_d.ap()[b, 128 * ct : 128 * ct + 128, qs],
                            )
                    for ct in range(2):
                        hs = slice(512, 1024)
                        qeng[ct].dma_start(
                            Xb[ct][:, hs], x_d.ap()[b, 128 * ct : 128 * ct + 128, hs]
                        )
                    nc.gpsimd.dma_start(
                        wqkvT[:, :, 256:512], wqkvT_d.ap()[:, :, 256:512]
                    )
                    nc.sync.dma_start(
                        ones512, ones_d.ap().rearrange("p (c d) -> p c d", c=8)
                    )
                else:
                    for ct in range(2):
                        nc.sync.dma_start(
                            Xb[ct], x_d.ap()[b, 128 * ct : 128 * ct + 128, :]
                        )
                Xall.append(Xb)

            # persistent vT tile: per head-pair strip [vT_e | 1 | vT_o]
            # (192 wide) so lhsT slices [vT_e|1] / [1|vT_o] stay contiguous.
            Vta = vtp.tile([128, 8, 384], DT_VT, name="vta", tag="vta")
            for hp in range(2):
                nc.vector.tensor_copy(
                    Vta[:, :, 192 * hp + 64 : 192 * hp + 128], ones512
                )

            # persistent padded-v images (bf16): border zeroed once; the qkv
            # V-evacuation writes straight into the interior, so no GpSimd
            # pad copies are needed.
            Pads = []
            for b in range(BL):
                pb = []
                for ct in range(2):
                    padt = padp.tile(
                        [128, 34, 34], DT_DW, name=f"pad{b}{ct}", tag=f"pad{b}{ct}"
                    )
                    nc.vector.memset(padt, 0.0)
                    pb.append(padt)
                Pads.append(pb)

            # ---- staged emission with cross-batch software pipelining ----
            st = [{} for _ in range(BL)]

            def emit_head(b):
                X = Xall[b]
                Qa = qkvp.tile([128, L], DT_QK, name=f"Qa_{b}", tag="Qa")
                Ka = qkvp.tile([128, L], DT_QK, name=f"Ka_{b}", tag="Ka")
                for mt in range(2):  # mt outer: consume x halves as they land
                    cw = 256 if (b == 0 and mt == 0) else 512
                    for ot in (1, 0, 2, 3):
                        for off in range(512 * mt, 512 * mt + 512, cw):
                            ms = slice(off, off + cw)
                            pq = pwork.tile(
                                [128, cw], F32, name=f"pq{b}{ot}{off}", tag="w"
                            )
                            for kt in range(2):
                                nc.tensor.matmul(
                                    pq,
                                    wqkvT[:, kt, 128 * ot : 128 * ot + 128],
                                    X[kt][:, ms],
                                    start=(kt == 0),
                                    stop=(kt == 1),
                                )
                            if ot < 2:
                                nc.vector.tensor_scalar(
                                    [Qa, Ka][ot][:, ms], pq, bqkv[:, ot : ot + 1],
                                    None, OP.add,
                                )
                            else:
                                # V: evacuate into the padded image interior
                                # (depthwise path; includes the v bias)
                                nr = cw // 32
                                nc.vector.tensor_scalar(
                                    Pads[b][ot - 2][
                                        :, 1 + off // 32 : 1 + off // 32 + nr, 1:33
                                    ],
                                    pq.rearrange("p (a c) -> p a c", a=nr),
                                    bqkv[:, ot : ot + 1],
                                    None, OP.add,
                                )
                if b == 0:
                    # late consts: needed only by dw/pw phases
                    nc.scalar.dma_start(wpwT, wpwT_d.ap())
                    nc.gpsimd.dma_start(bpw, bpw_d.ap())
                    nc.gpsimd.dma_start(diag, diag_d.ap())

                # vT computed directly: out[l, d] = sum_i x[i, l] WvT[i, d]
                # (x slices as stationary operand, V-weight columns as the
                # stream) — no PE transposes, no flat-V staging.  The v bias
                # is NOT added here; its (softmax-weights-sum-to-1) exact
                # contribution is folded into the pw bias on the host.
                for lb in range(8):
                    pv = pwork.tile([128, 256], F32, name=f"pv{b}{lb}", tag="w")
                    for kt in range(2):
                        nc.tensor.matmul(
                            pv,
                            X[kt][:, 128 * lb : 128 * lb + 128],
                            wqkvT[:, kt, 256:512],
                            start=(kt == 0),
                            stop=(kt == 1),
                        )
                    for hp in range(2):
                        dst = Vta[:, lb, 192 * hp : 192 * hp + 192].rearrange(
                            "p (s d) -> p s d", d=64
                        )[:, 0::2, :]
                        nc.vector.tensor_copy(
                            dst,
                            pv[:, 128 * hp : 128 * hp + 128].rearrange(
                                "p (s d) -> p s d", d=64
                            ),
                        )

                out2 = [
                    o2p.tile([128, L], DT_PW, name=f"o2_{b}{ct}", tag="o2")
                    for ct in range(2)
                ]
                st[b].update(Qa=Qa, Ka=Ka, pads=Pads[b], out2=out2)

            ID_MASK = list(range(32))

            def emit_attn(b, hp, mts=(0, 1), fine=False):
                # per (hp, mt): bank A = [vT_e|1].T @ E_e = [O_e; Z_e],
                # bank B = [1|vT_o].T @ E_o = [Z_o; O_o]; 1/Z via one fast
                # approx reciprocal; halves swapped with a DVE shuffle.
                Qa, Ka, out2 = st[b]["Qa"], st[b]["Ka"], st[b]["out2"]
                for mt in mts:
                    ms = slice(512 * mt, 512 * mt + 512)
                    pA = pOp.tile([128, 512], F32, name=f"pa{b}{hp}{mt}", tag="o")
                    pB = pOp.tile([128, 512], F32, name=f"pb{b}{hp}{mt}", tag="o")
                    banks = [pA, pB]
                    for lt in range(8):
                        ls = slice(128 * lt, 128 * lt + 128)
                        sc = psc.tile(
                            [128, 1024], F32, name=f"sc{b}{hp}{mt}{lt}", tag="sc"
                        )
                        for j in range(2):
                            h = 2 * hp + j
                            nc.tensor.matmul(
                                sc[:, 512 * j : 512 * j + 512],
                                Ka[32 * h : 32 * h + 32, ls],
                                Qa[32 * h : 32 * h + 32, ms],
                                start=True,
                                stop=True,
                                tile_position=(32 * h, 0),
                            )
                        Et = etp.tile(
                            [128, 1024], DT_VT, name=f"e{b}{hp}{mt}{lt}", tag="e"
                        )
                        nc.scalar.activation(Et, sc, AF.Exp)
                        for j in range(2):
                            nc.tensor.matmul(
                                banks[j],
                                Vta[:, lt, 192 * hp + 64 * j : 192 * hp + 64 * j + 128],
                                Et[:, 512 * j : 512 * j + 512],
                                start=(lt == 0),
                                stop=(lt == 7),
                                skip_group_check=True,
                            )
                    # stage Z (cross-half via shuffle-copy) and O halves to
                    # SBUF: ZA ends up swap-aligned with Ou, so no extra
                    # swap op is needed before the reciprocal.  `fine`
                    # chunks the chain to shorten tail latency.
                    steps = ((0, 256), (256, 256)) if fine else ((0, 512),)
                    for off, cw in steps:
                        cs = slice(off, off + cw)
                        os_ = slice(512 * mt + off, 512 * mt + off + cw)
                        ZA = smallp.tile(
                            [128, cw], F32, name=f"za{b}{hp}{mt}{off}", tag="za"
                        )
                        nc.vector.stream_shuffle(ZA[0:64, :], pA[64:128, cs], ID_MASK)
                        nc.vector.stream_shuffle(ZA[64:128, :], pB[0:64, cs], ID_MASK)
                        Rz = smallp.tile(
                            [128, cw], F32, name=f"rz{b}{hp}{mt}{off}", tag="rz"
                        )
                        nc.vector.reciprocal_approx_fast(out=Rz, in_=ZA)
                        # multiply O halves straight out of PSUM (no staging)
                        nc.vector.scalar_tensor_tensor(
                            out=out2[hp][0:64, os_],
                            in0=pA[0:64, cs],
                            scalar=1.0,
                            in1=Rz[0:64, :],
                            op0=OP.mult,
                            op1=OP.mult,
                        )
                        nc.vector.scalar_tensor_tensor(
                            out=out2[hp][64:128, os_],
                            in0=pB[64:128, cs],
                            scalar=1.0,
                            in1=Rz[64:128, :],
                            op0=OP.mult,
                            op1=OP.mult,
                        )

            def emit_dw(b, ct, mts=(0, 1), fine=False):
                padt, out2 = st[b]["pads"][ct], st[b]["out2"]
                for mt in mts:
                    dwp = pwork.tile([128, 512], F32, name=f"dw{b}{ct}{mt}", tag="w")
                    for tap in range(9):
                        dy, dx = tap // 3, tap % 3
                        r0 = 16 * mt + dy
                        nc.tensor.matmul(
                            dwp,
                            diag[:, 9 * ct + tap, :],
                            padt[:, r0 : r0 + 16, dx : dx + 32],
                            start=(tap == 0),
                            stop=(tap == 8),
                        )
                    steps = ((0, 256), (256, 256)) if fine else ((0, 512),)
                    for off, cw in steps:
                        os_ = slice(512 * mt + off, 512 * mt + off + cw)
                        nc.vector.scalar_tensor_tensor(
                            out=out2[ct][:, os_],
                            in0=dwp[:, off : off + cw],
                            scalar=1.0,
                            in1=out2[ct][:, os_],
                            op0=OP.mult,
                            op1=OP.add,
                        )

            def emit_pw(b, mt, fine=False):
                out2 = st[b]["out2"]
                steps = ((0, 256), (256, 256)) if fine else ((0, 512),)
                for off, cw in steps:
                    ms = slice(512 * mt + off, 512 * mt + off + cw)
                    for ot in range(2):
                        pp = pwork.tile(
                            [128, cw], F32, name=f"pp{b}{mt}{ot}{off}", tag="w"
                        )
                        for kt in range(2):
                            nc.tensor.matmul(
                                pp,
                                wpwT[:, kt, 128 * ot : 128 * ot + 128],
                                out2[kt][:, ms],
                                start=(kt == 0),
                                stop=(kt == 1),
                            )
                        osb = outp.tile(
                            [128, cw], F32, name=f"os{b}{mt}{ot}{off}", tag="os"
                        )
                        nc.scalar.activation(
                            osb, pp, AF.Identity, bias=bpw[:, ot : ot + 1]
                        )
                        nc.sync.dma_start(
                            out_d.ap()[b, 128 * ot : 128 * ot + 128, ms], osb
                        )

            assert BL == 2
            emit_head(0)
            emit_attn(0, 0)
            emit_dw(0, 0)
            emit_attn(0, 1)
            emit_head(1)          # batch-1 qkv before batch-0 tail work
            emit_dw(0, 1)
            emit_pw(0, 0)
            emit_pw(0, 1)
            emit_attn(1, 0)
            emit_dw(1, 0)
            emit_attn(1, 1, mts=(0,))
            emit_dw(1, 1, mts=(0,))
            emit_pw(1, 0)
            emit_attn(1, 1, mts=(1,), fine=True)
            emit_dw(1, 1, mts=(1,), fine=True)
            emit_pw(1, 1, fine=True)

    nc.compile()
    return nc


def pack_inputs(w_qkv, s_qkv, b_qkv, w_dw, s_dw, b_dw, w_pw, s_pw, b_pw):
    """Host-side weight packing. Returns dict of constant arrays (shared by
    all cores)."""
    f32 = np.float32
    Wq = (w_qkv[:, :, 0, 0] * s_qkv[:, None]).astype(np.float64)  # [512, 256]
    bq = b_qkv.astype(np.float64).copy()

    # permute output channels to [Q_all, K_all, V0, V1]
    perm = []
    for h in range(NH):
        perm += [h * 128 + d for d in range(32)]           # q
    for h in range(NH):
        perm += [h * 128 + 32 + d for d in range(32)]      # k
    for h in range(NH):
        perm += [h * 128 + 64 + d for d in range(64)]      # v
    perm = np.array(perm)
    Wq = Wq[perm]
    bq = bq[perm]
    # fold attention scale into q
    Wq[0:128] *= SCALE
    bq[0:128] *= SCALE

    wqkvT = np.ascontiguousarray(
        Wq.T.reshape(2, 128, CQKV).transpose(1, 0, 2)
    ).astype(__import__("ml_dtypes").bfloat16)  # [128, 2, 512]
    bqkv = np.ascontiguousarray(bq.reshape(4, 128).T).astype(f32)  # [128, 4]

    bf16_ = __import__("ml_dtypes").bfloat16
    Wp = (w_pw[:, :, 0, 0] * s_pw[:, None]).astype(np.float64)     # [256, 256]
    # b_dw fold (dw BN bias) + v-bias fold for the attention path (the
    # attention output is a softmax-weighted average, weights sum to 1, so
    # the missing per-channel v bias contributes exactly Wp @ bv).
    bv = bq[256:512]
    bp = b_pw.astype(np.float64) + Wp @ b_dw.astype(np.float64) + Wp @ bv
    wpwT = np.ascontiguousarray(
        Wp.T.reshape(2, 128, CH).transpose(1, 0, 2)
    ).astype(bf16_)  # [128, 2, 256]
    bpw = np.ascontiguousarray(bp.reshape(2, 128).T).astype(f32)   # [128, 2]

    bf16 = __import__("ml_dtypes").bfloat16
    wd = (w_dw[:, 0] * s_dw[:, None, None]).astype(f32)            # [256, 3, 3]
    diag = np.zeros((128, 18, 128), f32)
    for ct in range(2):
        for tap in range(9):
            dy, dx = tap // 3, tap % 3
            idx = np.arange(128)
            diag[idx, 9 * ct + tap, idx] = wd[128 * ct + idx, dy, dx]

    return {
        "wqkvT": wqkvT,
        "bqkv": bqkv,
        "wpwT": wpwT,
        "bpw": bpw,
        "diag": diag.astype(bf16),
        "ones512": np.ones((128, 512), bf16 if CFG["o_bf16"] else f32),
    }


_NC_CACHE = None


def _get_nc():
    global _NC_CACHE
    if _NC_CACHE is None:
        _NC_CACHE = build_bass()
    return _NC_CACHE


_RUNNER_CACHE = None


def _get_runner():
    """Cached jitted multi-core runner (mirrors bass2jax.run_bass_via_pjrt's
    multi-core path, but reuses the compiled executable across calls)."""
    global _RUNNER_CACHE
    if _RUNNER_CACHE is not None:
        return _RUNNER_CACHE

    import jax
    import jax.numpy as jnp
    from jax.experimental.shard_map import shard_map
    from jax.sharding import Mesh, PartitionSpec

    import concourse.mybir as mybir_
    from concourse import bass2jax

    nc = _get_nc()
    bass2jax.install_neuronx_cc_hook()

    in_names, out_names, out_avals, zero_outs = [], [], [], []
    for alloc in nc.m.functions[0].allocations:
        if not isinstance(mybir_.MemoryLocationSet, type) or not isinstance(
            alloc, mybir_.MemoryLocationSet
        ):
            continue
        name = alloc.memorylocations[0].name
        if alloc.kind == "ExternalInput":
            in_names.append(name)
        elif alloc.kind == "ExternalOutput":
            shape = tuple(alloc.tensor_shape)
            dtype = mybir_.dt.np(alloc.dtype)
            out_names.append(name)
            out_avals.append(jax.core.ShapedArray(shape, dtype))
            zero_outs.append(np.zeros(shape, dtype))
    n_params = len(in_names)
    n_outs = len(out_avals)
    all_in_names = list(in_names) + list(out_names)
    donate = tuple(range(n_params, n_params + n_outs))

    def _body(*args):
        outs = bass2jax._bass_exec_p.bind(
            *args,
            out_avals=tuple(out_avals),
            in_names=tuple(all_in_names),
            out_names=tuple(out_names),
            lowering_input_output_aliases=(),
            sim_require_finite=True,
            sim_require_nnan=True,
            nc=nc,
        )
        return tuple(outs)

    devices = jax.devices()[:NCORES]
    mesh = Mesh(np.asarray(devices), ("core",))
    sharded = jax.jit(
        shard_map(
            _body,
            mesh=mesh,
            in_specs=(PartitionSpec("core"),) * (n_params + n_outs),
            out_specs=(PartitionSpec("core"),) * n_outs,
            check_rep=False,
        ),
        donate_argnums=donate,
        keep_unused=True,
    )

    def runner(in_maps):
        concat_in = [
            np.concatenate([np.asarray(m[name]) for m in in_maps], axis=0)
            for name in in_names
        ]
        concat_zeros = [
            np.zeros((NCORES * z.shape[0], *z.shape[1:]), z.dtype) for z in zero_outs
        ]
        out_arrs = sharded(*concat_in, *concat_zeros)
        return [
            {
                name: np.asarray(out_arrs[i]).reshape(NCORES, *out_avals[i].shape)[c]
                for i, name in enumerate(out_names)
            }
            for c in range(NCORES)
        ]

    _RUNNER_CACHE = runner
    return runner


def run(inputs, trace=False):
    """Run the bass kernel on 8 cores. inputs = the reference input dict.
    Returns (full_output [16,256,32,32], BassKernelResults)."""
    x = np.ascontiguousarray(
        np.asarray(inputs["x"], dtype=np.float32)
        .astype(__import__("ml_dtypes").bfloat16)
    ).reshape(B, CH, L)
    consts = pack_inputs(
        np.asarray(inputs["w_qkv"], np.float32),
        np.asarray(inputs["s_qkv"], np.float32),
        np.asarray(inputs["b_qkv"], np.float32),
        np.asarray(inputs["w_dw"], np.float32),
        np.asarray(inputs["s_dw"], np.float32),
        np.asarray(inputs["b_dw"], np.float32),
        np.asarray(inputs["w_pw"], np.float32),
        np.asarray(inputs["s_pw"], np.float32),
        np.asarray(inputs["b_pw"], np.float32),
    )
    in_maps = []
    for c in range(NCORES):
        m = dict(consts)
        m["x"] = np.ascontiguousarray(x[c * BL : (c + 1) * BL])
        in_maps.append(m)

    nc = _get_nc()
    res = run_bass_kernel_spmd(
        nc, in_maps, core_ids=list(range(NCORES)), trace=trace
    )
    out = np.concatenate([r["out"] for r in res.results], axis=0)
    return out.reshape(B, CH, HH, WW), res


def kernel(**inputs) -> np.ndarray:
    out, _ = run(inputs, trace=False)
    return out



# revision 71
# speedup vs baseline: 1.0293x; 1.0293x over previous
"""Trainium2 Bass kernel for nn_Attention_50500225466997.

Computation (per batch): qkv = BN(conv1x1(x)); 4-head attention over L=1024
(DK=32, DH=64); out = attn + BN(dwconv3x3(v)); y = BN(conv1x1(out)).

Strategy:
  - Data-parallel over batch: 16 batches -> 8 NeuronCores, 2 per core.
  - All BN scales/permutations/SCALE folded into weights on the host.
  - Scores computed TRANSPOSED (S^T[l, m], l on partitions) so softmax
    needs no P-transposes: E = exp(S^T) unnormalized; Z rides FREE in the
    V-matmul by augmenting each head's vT lhsT with a 64-wide ones block
    ([vT_e|1] / [1|vT_o]) -> banks [O_e;Z_e] / [Z_o;O_o]; 1/Z via one
    exact DVE reciprocal, halves swapped by SBUF->SBUF DMA.
  - fp32r matmuls (fast fp32 streaming mode, ~tf32-accumulate-fp32).
  - Depthwise 3x3 via 9 diagonal-matrix matmuls accumulating in PSUM.
  - Cross-batch software pipelining: batch b+1's qkv/transposes emitted
    before batch b's depthwise/pointwise tail.
"""

import numpy as np

import concourse.bass as bass
import concourse.mybir as mybir
import concourse.tile as tile
from concourse import bacc
from concourse.bass_utils import run_bass_kernel_spmd

F32 = mybir.dt.float32
F32R = mybir.dt.float32r
AF = mybir.ActivationFunctionType
OP = mybir.AluOpType

B, CH, HH, WW = 16, 256, 32, 32
L = HH * WW                   # 1024
NH, DK, DH = 4, 32, 64
CQKV = CH + DK * NH * 2       # 512
SCALE = DK ** (-0.5)
NCORES = 8
BL = B // NCORES              # batches per core

# dtype config for precision experiments: flags switch a stage's matmul
# operands to full fp32 (4 cyc/row) instead of fp32r (1 cyc/row).
CFG = {"qkv32": False, "sc32": False, "o32": False, "pw32": False, "dw32": False, "o_bf16": True}




def build_bass():
    nc = bacc.Bacc("TRN2", target_bir_lowering=False, debug=False)
    BF16 = mybir.dt.bfloat16
    DT_XW = BF16
    DT_QK = BF16
    DT_VT = BF16 if CFG["o_bf16"] else (F32 if CFG["o32"] else F32R)
    DT_PW = BF16
    DT_DW = BF16

    x_d = nc.dram_tensor("x", [BL, CH, L], DT_XW, kind="ExternalInput")
    wqkvT_d = nc.dram_tensor("wqkvT", [128, 2, CQKV], DT_XW, kind="ExternalInput")
    bqkv_d = nc.dram_tensor("bqkv", [128, 4], F32, kind="ExternalInput")
    wpwT_d = nc.dram_tensor("wpwT", [128, 2, CH], DT_PW, kind="ExternalInput")
    bpw_d = nc.dram_tensor("bpw", [128, 2], F32, kind="ExternalInput")
    diag_d = nc.dram_tensor("diag", [128, 18, 128], DT_DW, kind="ExternalInput")
    id2_d = nc.dram_tensor("id2", [128, 64], BF16, kind="ExternalInput")
    ones_d = nc.dram_tensor("ones512", [128, 512], DT_VT, kind="ExternalInput")
    out_d = nc.dram_tensor("out", [BL, CH, L], F32, kind="ExternalOutput")

    with tile.TileContext(nc) as tc, nc.allow_low_precision(reason="fp32r"):
        with (
            tc.tile_pool(name="consts", bufs=1) as consts,
            tc.tile_pool(name="xin", bufs=4) as xin,
            tc.tile_pool(name="qkv", bufs=2) as qkvp,
            tc.tile_pool(name="vt", bufs=1) as vtp,
            tc.tile_pool(name="et", bufs=6) as etp,
            tc.tile_pool(name="o2", bufs=4) as o2p,
            tc.tile_pool(name="small", bufs=4) as smallp,
            tc.tile_pool(name="pad", bufs=4) as padp,
            tc.tile_pool(name="outp", bufs=4) as outp,
            tc.tile_pool(name="psc", bufs=2, space="PSUM") as psc,
            tc.tile_pool(name="pO", bufs=2, space="PSUM") as pOp,
            tc.tile_pool(name="pwork", bufs=2, space="PSUM") as pwork,
        ):
            # ---------------- constants (qkv path first) ----------------
            # DMA order tuned for startup latency: K/Q weights first, then
            # batch-0 x in half-tiles (first compute needs only the first
            # halves).  Startup DMAs are spread across issuing engines so
            # the per-DMA fixed overhead overlaps (separate HWDGE FIFOs).
            wqkvT = consts.tile([128, 2, CQKV], DT_XW)
            bqkv = consts.tile([128, 4], F32)
            nc.sync.dma_start(bqkv, bqkv_d.ap())
            nc.sync.dma_start(wqkvT[:, :, 0:256], wqkvT_d.ap()[:, :, 0:256])
            id2 = consts.tile([128, 64], BF16)
            ones512 = consts.tile([128, 8, 64], DT_VT)
            wpwT = consts.tile([128, 2, CH], DT_PW)
            bpw = consts.tile([128, 2], F32)
            diag = consts.tile([128, 18, 128], DT_DW)

            # prefetch all batches' x (xin bufs cover BL*2 tiles)
            Xall = []
            for b in range(BL):
                Xb = []
                for ct in range(2):
                    xt = xin.tile([128, L], DT_XW, name=f"x_b{b}c{ct}", tag="x")
                    Xb.append(xt)
                if b == 0:
                    # quarter-granularity for the first half (first qkv
                    # chunks start sooner), halves for the rest
                    qeng = [nc.scalar, nc.gpsimd]
                    for q in range(2):
                        for ct in range(2):
                            qs = slice(256 * q, 256 * q + 256)
                            qeng[ct].dma_start(
                                Xb[ct][:, qs],
                                x_d.ap()[b, 128 * ct : 128 * ct + 128, qs],
                            )
                    for ct in range(2):
                        hs = slice(512, 1024)
                        qeng[ct].dma_start(
                            Xb[ct][:, hs], x_d.ap()[b, 128 * ct : 128 * ct + 128, hs]
                        )
                    nc.gpsimd.dma_start(id2, id2_d.ap())
                    nc.gpsimd.dma_start(
                        wqkvT[:, :, 256:512], wqkvT_d.ap()[:, :, 256:512]
                    )
                    nc.sync.dma_start(
                        ones512, ones_d.ap().rearrange("p (c d) -> p c d", c=8)
                    )
                else:
                    for ct in range(2):
                        nc.sync.dma_start(
                            Xb[ct], x_d.ap()[b, 128 * ct : 128 * ct + 128, :]
                        )
                Xall.append(Xb)

            # persistent vt tiles: ones-halves written once
            Vt = []
            for h in range(NH):
                par = h % 2
                vt_h = vtp.tile([128, 8, 128], DT_VT, name=f"vt{h}", tag=f"vt{h}")
                nc.vector.tensor_copy(
                    vt_h[:, :, 64 - 64 * par : 128 - 64 * par], ones512
                )
                Vt.append(vt_h)

            # persistent padded-v images (bf16): border zeroed once; the qkv
            # V-evacuation writes straight into the interior, so no GpSimd
            # pad copies are needed.
            Pads = []
            for b in range(BL):
                pb = []
                for ct in range(2):
                    padt = padp.tile(
                        [128, 34, 34], DT_DW, name=f"pad{b}{ct}", tag=f"pad{b}{ct}"
                    )
                    nc.vector.memset(padt, 0.0)
                    pb.append(padt)
                Pads.append(pb)

            # ---- staged emission with cross-batch software pipelining ----
            st = [{} for _ in range(BL)]

            def emit_head(b):
                X = Xall[b]
                Qa = qkvp.tile([128, L], DT_QK, name=f"Qa_{b}", tag="Qa")
                Ka = qkvp.tile([128, L], DT_QK, name=f"Ka_{b}", tag="Ka")
                Vf = [
                    qkvp.tile([128, L], DT_VT, name=f"Vf_{b}{ct}", tag=f"Vf{ct}")
                    for ct in range(2)
                ]
                for mt in range(2):  # mt outer: consume x halves as they land
                    cw = 256 if (b == 0 and mt == 0) else 512
                    for ot in (1, 0, 2, 3):
                        for off in range(512 * mt, 512 * mt + 512, cw):
                            ms = slice(off, off + cw)
                            pq = pwork.tile(
                                [128, cw], F32, name=f"pq{b}{ot}{off}", tag="w"
                            )
                            for kt in range(2):
                                nc.tensor.matmul(
                                    pq,
                                    wqkvT[:, kt, 128 * ot : 128 * ot + 128],
                                    X[kt][:, ms],
                                    start=(kt == 0),
                                    stop=(kt == 1),
                                )
                            if ot < 2:
                                nc.vector.tensor_scalar(
                                    [Qa, Ka][ot][:, ms], pq, bqkv[:, ot : ot + 1],
                                    None, OP.add,
                                )
                            else:
                                # V: evacuate twice — padded image interior
                                # (DVE, depthwise) and flat (ACT, transposes).
                                nr = cw // 32
                                nc.vector.tensor_scalar(
                                    Pads[b][ot - 2][
                                        :, 1 + off // 32 : 1 + off // 32 + nr, 1:33
                                    ],
                                    pq.rearrange("p (a c) -> p a c", a=nr),
                                    bqkv[:, ot : ot + 1],
                                    None, OP.add,
                                )
                                nc.scalar.activation(
                                    Vf[ot - 2][:, ms], pq, AF.Identity,
                                    bias=bqkv[:, ot : ot + 1],
                                )
                if b == 0:
                    # late consts: needed only by dw/pw phases
                    nc.scalar.dma_start(wpwT, wpwT_d.ap())
                    nc.gpsimd.dma_start(bpw, bpw_d.ap())
                    nc.gpsimd.dma_start(diag, diag_d.ap())

                # vT transposes into the persistent augmented tiles
                for h in range(NH):
                    ct, lo = h // 2, (h % 2) * 64
                    pv = pwork.tile([128, 512], DT_VT, name=f"pv{b}{h}", tag="w")
                    for c8 in range(8):
                        nc.tensor.transpose(
                            pv[:, 64 * c8 : 64 * c8 + 64],
                            Vf[ct][lo : lo + 64, 128 * c8 : 128 * c8 + 128],
                            id2[lo : lo + 64, :],
                        )
                    par = h % 2
                    nc.vector.tensor_copy(
                        Vt[h][:, :, 64 * par : 64 * par + 64],
                        pv.rearrange("p (c d) -> p c d", c=8),
                    )

                out2 = [
                    o2p.tile([128, L], DT_PW, name=f"o2_{b}{ct}", tag="o2")
                    for ct in range(2)
                ]
                st[b].update(Qa=Qa, Ka=Ka, pads=Pads[b], out2=out2)

            ID_MASK = list(range(32))

            def emit_attn(b, hp, mts=(0, 1), fine=False):
                # per (hp, mt): bank A = [vT_e|1].T @ E_e = [O_e; Z_e],
                # bank B = [1|vT_o].T @ E_o = [Z_o; O_o]; 1/Z via one fast
                # approx reciprocal; halves swapped with a DVE shuffle.
                Qa, Ka, out2 = st[b]["Qa"], st[b]["Ka"], st[b]["out2"]
                for mt in mts:
                    ms = slice(512 * mt, 512 * mt + 512)
                    pA = pOp.tile([128, 512], F32, name=f"pa{b}{hp}{mt}", tag="o")
                    pB = pOp.tile([128, 512], F32, name=f"pb{b}{hp}{mt}", tag="o")
                    banks = [pA, pB]
                    for lt in range(8):
                        ls = slice(128 * lt, 128 * lt + 128)
                        sc = psc.tile(
                            [128, 1024], F32, name=f"sc{b}{hp}{mt}{lt}", tag="sc"
                        )
                        for j in range(2):
                            h = 2 * hp + j
                            nc.tensor.matmul(
                                sc[:, 512 * j : 512 * j + 512],
                                Ka[32 * h : 32 * h + 32, ls],
                                Qa[32 * h : 32 * h + 32, ms],
                                start=True,
                                stop=True,
                                tile_position=(32 * h, 0),
                            )
                        Et = etp.tile(
                            [128, 1024], DT_VT, name=f"e{b}{hp}{mt}{lt}", tag="e"
                        )
                        nc.scalar.activation(Et, sc, AF.Exp)
                        for j in range(2):
                            h = 2 * hp + j
                            nc.tensor.matmul(
                                banks[j],
                                Vt[h][:, lt, :],
                                Et[:, 512 * j : 512 * j + 512],
                                start=(lt == 0),
                                stop=(lt == 7),
                                skip_group_check=True,
                            )
                    # stage Z (cross-half via shuffle-copy) and O halves to
                    # SBUF: ZA ends up swap-aligned with Ou, so no extra
                    # swap op is needed before the reciprocal.  `fine`
                    # chunks the chain to shorten tail latency.
                    steps = ((0, 256), (256, 256)) if fine else ((0, 512),)
                    for off, cw in steps:
                        cs = slice(off, off + cw)
                        os_ = slice(512 * mt + off, 512 * mt + off + cw)
                        ZA = smallp.tile(
                            [128, cw], F32, name=f"za{b}{hp}{mt}{off}", tag="za"
                        )
                        nc.vector.stream_shuffle(ZA[0:64, :], pA[64:128, cs], ID_MASK)
                        nc.vector.stream_shuffle(ZA[64:128, :], pB[0:64, cs], ID_MASK)
                        Rz = smallp.tile(
                            [128, cw], F32, name=f"rz{b}{hp}{mt}{off}", tag="rz"
                        )
                        nc.vector.reciprocal_approx_fast(out=Rz, in_=ZA)
                        # multiply O halves straight out of PSUM (no staging)
                        nc.vector.scalar_tensor_tensor(
                            out=out2[hp][0:64, os_],
                            in0=pA[0:64, cs],
                            scalar=1.0,
                            in1=Rz[0:64, :],
                            op0=OP.mult,
                            op1=OP.mult,
                        )
                        nc.vector.scalar_tensor_tensor(
                            out=out2[hp][64:128, os_],
                            in0=pB[64:128, cs],
                            scalar=1.0,
                            in1=Rz[64:128, :],
                            op0=OP.mult,
                            op1=OP.mult,
                        )

            def emit_dw(b, ct, mts=(0, 1), fine=False):
                padt, out2 = st[b]["pads"][ct], st[b]["out2"]
                for mt in mts:
                    dwp = pwork.tile([128, 512], F32, name=f"dw{b}{ct}{mt}", tag="w")
                    for tap in range(9):
                        dy, dx = tap // 3, tap % 3
                        r0 = 16 * mt + dy
                        nc.tensor.matmul(
                            dwp,
                            diag[:, 9 * ct + tap, :],
                            padt[:, r0 : r0 + 16, dx : dx + 32],
                            start=(tap == 0),
                            stop=(tap == 8),
                        )
                    steps = ((0, 256), (256, 256)) if fine else ((0, 512),)
                    for off, cw in steps:
                        os_ = slice(512 * mt + off, 512 * mt + off + cw)
                        nc.vector.scalar_tensor_tensor(
                            out=out2[ct][:, os_],
                            in0=dwp[:, off : off + cw],
                            scalar=1.0,
                            in1=out2[ct][:, os_],
                            op0=OP.mult,
                            op1=OP.add,
                        )

            def emit_pw(b, mt, fine=False):
                out2 = st[b]["out2"]
                steps = ((0, 256), (256, 256)) if fine else ((0, 512),)
                for off, cw in steps:
                    ms = slice(512 * mt + off, 512 * mt + off + cw)
                    for ot in range(2):
                        pp = pwork.tile(
                            [128, cw], F32, name=f"pp{b}{mt}{ot}{off}", tag="w"
                        )
                        for kt in range(2):
                            nc.tensor.matmul(
                                pp,
                                wpwT[:, kt, 128 * ot : 128 * ot + 128],
                                out2[kt][:, ms],
                                start=(kt == 0),
                                stop=(kt == 1),
                            )
                        osb = outp.tile(
                            [128, cw], F32, name=f"os{b}{mt}{ot}{off}", tag="os"
                        )
                        nc.scalar.activation(
                            osb, pp, AF.Identity, bias=bpw[:, ot : ot + 1]
                        )
                        nc.sync.dma_start(
                            out_d.ap()[b, 128 * ot : 128 * ot + 128, ms], osb
                        )

            assert BL == 2
            emit_head(0)
            emit_attn(0, 0)
            emit_dw(0, 0)
            emit_attn(0, 1)
            emit_head(1)          # batch-1 qkv before batch-0 tail work
            emit_dw(0, 1)
            emit_pw(0, 0)
            emit_pw(0, 1)
            emit_attn(1, 0)
            emit_dw(1, 0)
            emit_attn(1, 1, mts=(0,))
            emit_dw(1, 1, mts=(0,))
            emit_pw(1, 0)
            emit_attn(1, 1, mts=(1,), fine=True)
            emit_dw(1, 1, mts=(1,), fine=True)
            emit_pw(1, 1, fine=True)

    nc.compile()
    return nc


def pack_inputs(w_qkv, s_qkv, b_qkv, w_dw, s_dw, b_dw, w_pw, s_pw, b_pw):
    """Host-side weight packing. Returns dict of constant arrays (shared by
    all cores)."""
    f32 = np.float32
    Wq = (w_qkv[:, :, 0, 0] * s_qkv[:, None]).astype(np.float64)  # [512, 256]
    bq = b_qkv.astype(np.float64).copy()

    # permute output channels to [Q_all, K_all, V0, V1]
    perm = []
    for h in range(NH):
        perm += [h * 128 + d for d in range(32)]           # q
    for h in range(NH):
        perm += [h * 128 + 32 + d for d in range(32)]      # k
    for h in range(NH):
        perm += [h * 128 + 64 + d for d in range(64)]      # v
    perm = np.array(perm)
    Wq = Wq[perm]
    bq = bq[perm]
    # fold attention scale into q
    Wq[0:128] *= SCALE
    bq[0:128] *= SCALE

    wqkvT = np.ascontiguousarray(
        Wq.T.reshape(2, 128, CQKV).transpose(1, 0, 2)
    ).astype(__import__("ml_dtypes").bfloat16)  # [128, 2, 512]
    bqkv = np.ascontiguousarray(bq.reshape(4, 128).T).astype(f32)  # [128, 4]

    bf16_ = __import__("ml_dtypes").bfloat16
    Wp = (w_pw[:, :, 0, 0] * s_pw[:, None]).astype(np.float64)     # [256, 256]
    bp = b_pw.astype(np.float64) + Wp @ b_dw.astype(np.float64)
    wpwT = np.ascontiguousarray(
        Wp.T.reshape(2, 128, CH).transpose(1, 0, 2)
    ).astype(bf16_)  # [128, 2, 256]
    bpw = np.ascontiguousarray(bp.reshape(2, 128).T).astype(f32)   # [128, 2]

    bf16 = __import__("ml_dtypes").bfloat16
    wd = (w_dw[:, 0] * s_dw[:, None, None]).astype(f32)            # [256, 3, 3]
    diag = np.zeros((128, 18, 128), f32)
    for ct in range(2):
        for tap in range(9):
            dy, dx = tap // 3, tap % 3
            idx = np.arange(128)
            diag[idx, 9 * ct + tap, idx] = wd[128 * ct + idx, dy, dx]

    id2 = np.tile(np.eye(64, dtype=f32), (2, 1))                   # [128, 64]

    return {
        "wqkvT": wqkvT,
        "bqkv": bqkv,
        "wpwT": wpwT,
        "bpw": bpw,
        "diag": diag.astype(bf16),
        "id2": id2.astype(bf16),
        "ones512": np.ones((128, 512), bf16 if CFG["o_bf16"] else f32),
    }


_NC_CACHE = None


def _get_nc():
    global _NC_CACHE
    if _NC_CACHE is None:
        _NC_CACHE = build_bass()
    return _NC_CACHE


_RUNNER_CACHE = None


def _get_runner():
    """Cached jitted multi-core runner (mirrors bass2jax.run_bass_via_pjrt's
    multi-core path, but reuses the compiled executable across calls)."""
    global _RUNNER_CACHE
    if _RUNNER_CACHE is not None:
        return _RUNNER_CACHE

    import jax
    import jax.numpy as jnp
    from jax.experimental.shard_map import shard_map
    from jax.sharding import Mesh, PartitionSpec

    import concourse.mybir as mybir_
    from concourse import bass2jax

    nc = _get_nc()
    bass2jax.install_neuronx_cc_hook()

    in_names, out_names, out_avals, zero_outs = [], [], [], []
    for alloc in nc.m.functions[0].allocations:
        if not isinstance(mybir_.MemoryLocationSet, type) or not isinstance(
            alloc, mybir_.MemoryLocationSet
        ):
            continue
        name = alloc.memorylocations[0].name
        if alloc.kind == "ExternalInput":
            in_names.append(name)
        elif alloc.kind == "ExternalOutput":
            shape = tuple(alloc.tensor_shape)
            dtype = mybir_.dt.np(alloc.dtype)
            out_names.append(name)
            out_avals.append(jax.core.ShapedArray(shape, dtype))
            zero_outs.append(np.zeros(shape, dtype))
    n_params = len(in_names)
    n_outs = len(out_avals)
    all_in_names = list(in_names) + list(out_names)
    donate = tuple(range(n_params, n_params + n_outs))

    def _body(*args):
        outs = bass2jax._bass_exec_p.bind(
            *args,
            out_avals=tuple(out_avals),
            in_names=tuple(all_in_names),
            out_names=tuple(out_names),
            lowering_input_output_aliases=(),
            sim_require_finite=True,
            sim_require_nnan=True,
            nc=nc,
        )
        return tuple(outs)

    devices = jax.devices()[:NCORES]
    mesh = Mesh(np.asarray(devices), ("core",))
    sharded = jax.jit(
        shard_map(
            _body,
            mesh=mesh,
            in_specs=(PartitionSpec("core"),) * (n_params + n_outs),
            out_specs=(PartitionSpec("core"),) * n_outs,
            check_rep=False,
        ),
        donate_argnums=donate,
        keep_unused=True,
    )

    def runner(in_maps):
        concat_in = [
            np.concatenate([np.asarray(m[name]) for m in in_maps], axis=0)
            for name in in_names
        ]
        concat_zeros = [
            np.zeros((NCORES * z.shape[0], *z.shape[1:]), z.dtype) for z in zero_outs
        ]
        out_arrs = sharded(*concat_in, *concat_zeros)
        return [
            {
                name: np.asarray(out_arrs[i]).reshape(NCORES, *out_avals[i].shape)[c]
                for i, name in enumerate(out_names)
            }
            for c in range(NCORES)
        ]

    _RUNNER_CACHE = runner
    return runner


def run(inputs, trace=False):
    """Run the bass kernel on 8 cores. inputs = the reference input dict.
    Returns (full_output [16,256,32,32], BassKernelResults)."""
    x = np.ascontiguousarray(
        np.asarray(inputs["x"], dtype=np.float32)
        .astype(__import__("ml_dtypes").bfloat16)
    ).reshape(B, CH, L)
    consts = pack_inputs(
        np.asarray(inputs["w_qkv"], np.float32),
        np.asarray(inputs["s_qkv"], np.float32),
        np.asarray(inputs["b_qkv"], np.float32),
        np.asarray(inputs["w_dw"], np.float32),
        np.asarray(inputs["s_dw"], np.float32),
        np.asarray(inputs["b_dw"], np.float32),
        np.asarray(inputs["w_pw"], np.float32),
        np.asarray(inputs["s_pw"], np.float32),
        np.asarray(inputs["b_pw"], np.float32),
    )
    in_maps = []
    for c in range(NCORES):
        m = dict(consts)
        m["x"] = np.ascontiguousarray(x[c * BL : (c + 1) * BL])
        in_maps.append(m)

    nc = _get_nc()
    res = run_bass_kernel_spmd(
        nc, in_maps, core_ids=list(range(NCORES)), trace=trace
    )
    out = np.concatenate([r["out"] for r in res.results], axis=0)
    return out.reshape(B, CH, HH, WW), res


def kernel(**inputs) -> np.ndarray:
    out, _ = run(inputs, trace=False)
    return out



# revision 73
# speedup vs baseline: 1.0488x; 1.0189x over previous
"""Trainium2 Bass kernel for nn_Attention_50500225466997.

Computation (per batch): qkv = BN(conv1x1(x)); 4-head attention over L=1024
(DK=32, DH=64); out = attn + BN(dwconv3x3(v)); y = BN(conv1x1(out)).

Strategy:
  - Data-parallel over batch: 16 batches -> 8 NeuronCores, 2 per core.
  - All BN scales/permutations/SCALE folded into weights on the host.
  - Scores computed TRANSPOSED (S^T[l, m], l on partitions) so softmax
    needs no P-transposes: E = exp(S^T) unnormalized; Z rides FREE in the
    V-matmul by augmenting each head's vT lhsT with a 64-wide ones block
    ([vT_e|1] / [1|vT_o]) -> banks [O_e;Z_e] / [Z_o;O_o]; 1/Z via one
    exact DVE reciprocal, halves swapped by SBUF->SBUF DMA.
  - fp32r matmuls (fast fp32 streaming mode, ~tf32-accumulate-fp32).
  - Depthwise 3x3 via 9 diagonal-matrix matmuls accumulating in PSUM.
  - Cross-batch software pipelining: batch b+1's qkv/transposes emitted
    before batch b's depthwise/pointwise tail.
"""

import numpy as np

import concourse.bass as bass
import concourse.mybir as mybir
import concourse.tile as tile
from concourse import bacc
from concourse.bass_utils import run_bass_kernel_spmd

F32 = mybir.dt.float32
F32R = mybir.dt.float32r
AF = mybir.ActivationFunctionType
OP = mybir.AluOpType

B, CH, HH, WW = 16, 256, 32, 32
L = HH * WW                   # 1024
NH, DK, DH = 4, 32, 64
CQKV = CH + DK * NH * 2       # 512
SCALE = DK ** (-0.5)
NCORES = 8
BL = B // NCORES              # batches per core

# dtype config for precision experiments: flags switch a stage's matmul
# operands to full fp32 (4 cyc/row) instead of fp32r (1 cyc/row).
CFG = {"qkv32": False, "sc32": False, "o32": False, "pw32": False, "dw32": False, "o_bf16": True}




def build_bass():
    nc = bacc.Bacc("TRN2", target_bir_lowering=False, debug=False)
    BF16 = mybir.dt.bfloat16
    DT_XW = BF16
    DT_QK = BF16
    DT_VT = BF16 if CFG["o_bf16"] else (F32 if CFG["o32"] else F32R)
    DT_PW = BF16
    DT_DW = BF16

    x_d = nc.dram_tensor("x", [BL, CH, L], DT_XW, kind="ExternalInput")
    wqkvT_d = nc.dram_tensor("wqkvT", [128, 2, CQKV], DT_XW, kind="ExternalInput")
    bqkv_d = nc.dram_tensor("bqkv", [128, 4], F32, kind="ExternalInput")
    wpwT_d = nc.dram_tensor("wpwT", [128, 2, CH], DT_PW, kind="ExternalInput")
    bpw_d = nc.dram_tensor("bpw", [128, 2], F32, kind="ExternalInput")
    diag_d = nc.dram_tensor("diag", [128, 18, 128], DT_DW, kind="ExternalInput")
    id2_d = nc.dram_tensor("id2", [128, 64], BF16, kind="ExternalInput")
    ones_d = nc.dram_tensor("ones512", [128, 512], DT_VT, kind="ExternalInput")
    out_d = nc.dram_tensor("out", [BL, CH, L], F32, kind="ExternalOutput")

    with tile.TileContext(nc) as tc, nc.allow_low_precision(reason="fp32r"):
        with (
            tc.tile_pool(name="consts", bufs=1) as consts,
            tc.tile_pool(name="xin", bufs=4) as xin,
            tc.tile_pool(name="qkv", bufs=2) as qkvp,
            tc.tile_pool(name="vt", bufs=1) as vtp,
            tc.tile_pool(name="et", bufs=6) as etp,
            tc.tile_pool(name="o2", bufs=4) as o2p,
            tc.tile_pool(name="small", bufs=4) as smallp,
            tc.tile_pool(name="pad", bufs=4) as padp,
            tc.tile_pool(name="outp", bufs=4) as outp,
            tc.tile_pool(name="psc", bufs=2, space="PSUM") as psc,
            tc.tile_pool(name="pO", bufs=2, space="PSUM") as pOp,
            tc.tile_pool(name="pwork", bufs=2, space="PSUM") as pwork,
        ):
            # ---------------- constants (qkv path first) ----------------
            # DMA order tuned for startup latency: K/Q weights first, then
            # batch-0 x in half-tiles (first compute needs only the first
            # halves).  Startup DMAs are spread across issuing engines so
            # the per-DMA fixed overhead overlaps (separate HWDGE FIFOs).
            wqkvT = consts.tile([128, 2, CQKV], DT_XW)
            bqkv = consts.tile([128, 4], F32)
            nc.sync.dma_start(bqkv, bqkv_d.ap())
            nc.sync.dma_start(wqkvT[:, :, 0:256], wqkvT_d.ap()[:, :, 0:256])
            id2 = consts.tile([128, 64], BF16)
            ones512 = consts.tile([128, 8, 64], DT_VT)
            wpwT = consts.tile([128, 2, CH], DT_PW)
            bpw = consts.tile([128, 2], F32)
            diag = consts.tile([128, 18, 128], DT_DW)

            # prefetch all batches' x (xin bufs cover BL*2 tiles)
            Xall = []
            for b in range(BL):
                Xb = []
                for ct in range(2):
                    xt = xin.tile([128, L], DT_XW, name=f"x_b{b}c{ct}", tag="x")
                    Xb.append(xt)
                if b == 0:
                    # quarter-granularity for the first half (first qkv
                    # chunks start sooner), halves for the rest
                    qeng = [nc.scalar, nc.gpsimd]
                    for q in range(2):
                        for ct in range(2):
                            qs = slice(256 * q, 256 * q + 256)
                            qeng[ct].dma_start(
                                Xb[ct][:, qs],
                                x_d.ap()[b, 128 * ct : 128 * ct + 128, qs],
                            )
                    for ct in range(2):
                        hs = slice(512, 1024)
                        qeng[ct].dma_start(
                            Xb[ct][:, hs], x_d.ap()[b, 128 * ct : 128 * ct + 128, hs]
                        )
                    nc.gpsimd.dma_start(id2, id2_d.ap())
                    nc.gpsimd.dma_start(
                        wqkvT[:, :, 256:512], wqkvT_d.ap()[:, :, 256:512]
                    )
                    nc.sync.dma_start(
                        ones512, ones_d.ap().rearrange("p (c d) -> p c d", c=8)
                    )
                else:
                    for ct in range(2):
                        nc.sync.dma_start(
                            Xb[ct], x_d.ap()[b, 128 * ct : 128 * ct + 128, :]
                        )
                Xall.append(Xb)

            # persistent vt tiles: ones-halves written once
            Vt = []
            for h in range(NH):
                par = h % 2
                vt_h = vtp.tile([128, 8, 128], DT_VT, name=f"vt{h}", tag=f"vt{h}")
                nc.vector.tensor_copy(
                    vt_h[:, :, 64 - 64 * par : 128 - 64 * par], ones512
                )
                Vt.append(vt_h)

            # persistent padded-v images (bf16): border zeroed once; the qkv
            # V-evacuation writes straight into the interior, so no GpSimd
            # pad copies are needed.
            Pads = []
            for b in range(BL):
                pb = []
                for ct in range(2):
                    padt = padp.tile(
                        [128, 34, 34], DT_DW, name=f"pad{b}{ct}", tag=f"pad{b}{ct}"
                    )
                    nc.vector.memset(padt, 0.0)
                    pb.append(padt)
                Pads.append(pb)

            # ---- staged emission with cross-batch software pipelining ----
            st = [{} for _ in range(BL)]

            def emit_head(b):
                X = Xall[b]
                Qa = qkvp.tile([128, L], DT_QK, name=f"Qa_{b}", tag="Qa")
                Ka = qkvp.tile([128, L], DT_QK, name=f"Ka_{b}", tag="Ka")
                Vf = [
                    qkvp.tile([128, L], DT_VT, name=f"Vf_{b}{ct}", tag=f"Vf{ct}")
                    for ct in range(2)
                ]
                for mt in range(2):  # mt outer: consume x halves as they land
                    cw = 256 if (b == 0 and mt == 0) else 512
                    for ot in (1, 0, 2, 3):
                        for off in range(512 * mt, 512 * mt + 512, cw):
                            ms = slice(off, off + cw)
                            pq = pwork.tile(
                                [128, cw], F32, name=f"pq{b}{ot}{off}", tag="w"
                            )
                            for kt in range(2):
                                nc.tensor.matmul(
                                    pq,
                                    wqkvT[:, kt, 128 * ot : 128 * ot + 128],
                                    X[kt][:, ms],
                                    start=(kt == 0),
                                    stop=(kt == 1),
                                )
                            if ot < 2:
                                nc.vector.tensor_scalar(
                                    [Qa, Ka][ot][:, ms], pq, bqkv[:, ot : ot + 1],
                                    None, OP.add,
                                )
                            else:
                                # V: evacuate twice — padded image interior
                                # (DVE, depthwise) and flat (ACT, transposes).
                                nr = cw // 32
                                nc.vector.tensor_scalar(
                                    Pads[b][ot - 2][
                                        :, 1 + off // 32 : 1 + off // 32 + nr, 1:33
                                    ],
                                    pq.rearrange("p (a c) -> p a c", a=nr),
                                    bqkv[:, ot : ot + 1],
                                    None, OP.add,
                                )
                                nc.scalar.activation(
                                    Vf[ot - 2][:, ms], pq, AF.Identity,
                                    bias=bqkv[:, ot : ot + 1],
                                )
                if b == 0:
                    # late consts: needed only by dw/pw phases
                    nc.scalar.dma_start(wpwT, wpwT_d.ap())
                    nc.gpsimd.dma_start(bpw, bpw_d.ap())
                    nc.gpsimd.dma_start(diag, diag_d.ap())

                # vT transposes into the persistent augmented tiles
                for h in range(NH):
                    ct, lo = h // 2, (h % 2) * 64
                    pv = pwork.tile([128, 512], DT_VT, name=f"pv{b}{h}", tag="w")
                    for c8 in range(8):
                        nc.tensor.transpose(
                            pv[:, 64 * c8 : 64 * c8 + 64],
                            Vf[ct][lo : lo + 64, 128 * c8 : 128 * c8 + 128],
                            id2[lo : lo + 64, :],
                        )
                    par = h % 2
                    nc.vector.tensor_copy(
                        Vt[h][:, :, 64 * par : 64 * par + 64],
                        pv.rearrange("p (c d) -> p c d", c=8),
                    )

                out2 = [
                    o2p.tile([128, L], DT_PW, name=f"o2_{b}{ct}", tag="o2")
                    for ct in range(2)
                ]
                st[b].update(Qa=Qa, Ka=Ka, pads=Pads[b], out2=out2)

            ID_MASK = list(range(32))

            def emit_attn(b, hp, mts=(0, 1), fine=False):
                # per (hp, mt): bank A = [vT_e|1].T @ E_e = [O_e; Z_e],
                # bank B = [1|vT_o].T @ E_o = [Z_o; O_o]; 1/Z via one fast
                # approx reciprocal; halves swapped with a DVE shuffle.
                Qa, Ka, out2 = st[b]["Qa"], st[b]["Ka"], st[b]["out2"]
                for mt in mts:
                    ms = slice(512 * mt, 512 * mt + 512)
                    pA = pOp.tile([128, 512], F32, name=f"pa{b}{hp}{mt}", tag="o")
                    pB = pOp.tile([128, 512], F32, name=f"pb{b}{hp}{mt}", tag="o")
                    banks = [pA, pB]
                    for lt in range(8):
                        ls = slice(128 * lt, 128 * lt + 128)
                        sc = psc.tile(
                            [128, 1024], F32, name=f"sc{b}{hp}{mt}{lt}", tag="sc"
                        )
                        for j in range(2):
                            h = 2 * hp + j
                            nc.tensor.matmul(
                                sc[:, 512 * j : 512 * j + 512],
                                Ka[32 * h : 32 * h + 32, ls],
                                Qa[32 * h : 32 * h + 32, ms],
                                start=True,
                                stop=True,
                                tile_position=(32 * h, 0),
                            )
                        Et = etp.tile(
                            [128, 1024], DT_VT, name=f"e{b}{hp}{mt}{lt}", tag="e"
                        )
                        nc.scalar.activation(Et, sc, AF.Exp)
                        for j in range(2):
                            h = 2 * hp + j
                            nc.tensor.matmul(
                                banks[j],
                                Vt[h][:, lt, :],
                                Et[:, 512 * j : 512 * j + 512],
                                start=(lt == 0),
                                stop=(lt == 7),
                                skip_group_check=True,
                            )
                    # stage Z (cross-half via shuffle-copy) and O halves to
                    # SBUF: ZA ends up swap-aligned with Ou, so no extra
                    # swap op is needed before the reciprocal.  `fine`
                    # chunks the chain to shorten tail latency.
                    steps = ((0, 256), (256, 256)) if fine else ((0, 512),)
                    for off, cw in steps:
                        cs = slice(off, off + cw)
                        os_ = slice(512 * mt + off, 512 * mt + off + cw)
                        ZA = smallp.tile(
                            [128, cw], F32, name=f"za{b}{hp}{mt}{off}", tag="za"
                        )
                        nc.vector.stream_shuffle(ZA[0:64, :], pA[64:128, cs], ID_MASK)
                        nc.vector.stream_shuffle(ZA[64:128, :], pB[0:64, cs], ID_MASK)
                        Ou = smallp.tile(
                            [128, cw], F32, name=f"ou{b}{hp}{mt}{off}", tag="ou"
                        )
                        nc.vector.tensor_copy(Ou[0:64, :], pA[0:64, cs])
                        nc.vector.tensor_copy(Ou[64:128, :], pB[64:128, cs])
                        Rz = smallp.tile(
                            [128, cw], F32, name=f"rz{b}{hp}{mt}{off}", tag="rz"
                        )
                        nc.vector.reciprocal_approx_fast(out=Rz, in_=ZA)
                        nc.vector.scalar_tensor_tensor(
                            out=out2[hp][:, os_],
                            in0=Ou,
                            scalar=1.0,
                            in1=Rz,
                            op0=OP.mult,
                            op1=OP.mult,
                        )

            def emit_dw(b, ct, mts=(0, 1), fine=False):
                padt, out2 = st[b]["pads"][ct], st[b]["out2"]
                for mt in mts:
                    dwp = pwork.tile([128, 512], F32, name=f"dw{b}{ct}{mt}", tag="w")
                    for tap in range(9):
                        dy, dx = tap // 3, tap % 3
                        r0 = 16 * mt + dy
                        nc.tensor.matmul(
                            dwp,
                            diag[:, 9 * ct + tap, :],
                            padt[:, r0 : r0 + 16, dx : dx + 32],
                            start=(tap == 0),
                            stop=(tap == 8),
                        )
                    steps = ((0, 256), (256, 256)) if fine else ((0, 512),)
                    for off, cw in steps:
                        os_ = slice(512 * mt + off, 512 * mt + off + cw)
                        nc.vector.scalar_tensor_tensor(
                            out=out2[ct][:, os_],
                            in0=dwp[:, off : off + cw],
                            scalar=1.0,
                            in1=out2[ct][:, os_],
                            op0=OP.mult,
                            op1=OP.add,
                        )

            def emit_pw(b, mt, fine=False):
                out2 = st[b]["out2"]
                steps = ((0, 256), (256, 256)) if fine else ((0, 512),)
                for off, cw in steps:
                    ms = slice(512 * mt + off, 512 * mt + off + cw)
                    for ot in range(2):
                        pp = pwork.tile(
                            [128, cw], F32, name=f"pp{b}{mt}{ot}{off}", tag="w"
                        )
                        for kt in range(2):
                            nc.tensor.matmul(
                                pp,
                                wpwT[:, kt, 128 * ot : 128 * ot + 128],
                                out2[kt][:, ms],
                                start=(kt == 0),
                                stop=(kt == 1),
                            )
                        osb = outp.tile(
                            [128, cw], F32, name=f"os{b}{mt}{ot}{off}", tag="os"
                        )
                        nc.vector.tensor_scalar(
                            osb, pp, bpw[:, ot : ot + 1], None, OP.add
                        )
                        nc.sync.dma_start(
                            out_d.ap()[b, 128 * ot : 128 * ot + 128, ms], osb
                        )

            assert BL == 2
            emit_head(0)
            emit_attn(0, 0)
            emit_dw(0, 0)
            emit_attn(0, 1)
            emit_head(1)          # batch-1 qkv before batch-0 tail work
            emit_dw(0, 1)
            emit_pw(0, 0)
            emit_pw(0, 1)
            emit_attn(1, 0)
            emit_dw(1, 0)
            emit_attn(1, 1, mts=(0,))
            emit_dw(1, 1, mts=(0,))
            emit_pw(1, 0)
            emit_attn(1, 1, mts=(1,), fine=True)
            emit_dw(1, 1, mts=(1,), fine=True)
            emit_pw(1, 1, fine=True)

    nc.compile()
    return nc


def pack_inputs(w_qkv, s_qkv, b_qkv, w_dw, s_dw, b_dw, w_pw, s_pw, b_pw):
    """Host-side weight packing. Returns dict of constant arrays (shared by
    all cores)."""
    f32 = np.float32
    Wq = (w_qkv[:, :, 0, 0] * s_qkv[:, None]).astype(np.float64)  # [512, 256]
    bq = b_qkv.astype(np.float64).copy()

    # permute output channels to [Q_all, K_all, V0, V1]
    perm = []
    for h in range(NH):
        perm += [h * 128 + d for d in range(32)]           # q
    for h in range(NH):
        perm += [h * 128 + 32 + d for d in range(32)]      # k
    for h in range(NH):
        perm += [h * 128 + 64 + d for d in range(64)]      # v
    perm = np.array(perm)
    Wq = Wq[perm]
    bq = bq[perm]
    # fold attention scale into q
    Wq[0:128] *= SCALE
    bq[0:128] *= SCALE

    wqkvT = np.ascontiguousarray(
        Wq.T.reshape(2, 128, CQKV).transpose(1, 0, 2)
    ).astype(__import__("ml_dtypes").bfloat16)  # [128, 2, 512]
    bqkv = np.ascontiguousarray(bq.reshape(4, 128).T).astype(f32)  # [128, 4]

    bf16_ = __import__("ml_dtypes").bfloat16
    Wp = (w_pw[:, :, 0, 0] * s_pw[:, None]).astype(np.float64)     # [256, 256]
    bp = b_pw.astype(np.float64) + Wp @ b_dw.astype(np.float64)
    wpwT = np.ascontiguousarray(
        Wp.T.reshape(2, 128, CH).transpose(1, 0, 2)
    ).astype(bf16_)  # [128, 2, 256]
    bpw = np.ascontiguousarray(bp.reshape(2, 128).T).astype(f32)   # [128, 2]

    bf16 = __import__("ml_dtypes").bfloat16
    wd = (w_dw[:, 0] * s_dw[:, None, None]).astype(f32)            # [256, 3, 3]
    diag = np.zeros((128, 18, 128), f32)
    for ct in range(2):
        for tap in range(9):
            dy, dx = tap // 3, tap % 3
            idx = np.arange(128)
            diag[idx, 9 * ct + tap, idx] = wd[128 * ct + idx, dy, dx]

    id2 = np.tile(np.eye(64, dtype=f32), (2, 1))                   # [128, 64]

    return {
        "wqkvT": wqkvT,
        "bqkv": bqkv,
        "wpwT": wpwT,
        "bpw": bpw,
        "diag": diag.astype(bf16),
        "id2": id2.astype(bf16),
        "ones512": np.ones((128, 512), bf16 if CFG["o_bf16"] else f32),
    }


_NC_CACHE = None


def _get_nc():
    global _NC_CACHE
    if _NC_CACHE is None:
        _NC_CACHE = build_bass()
    return _NC_CACHE


_RUNNER_CACHE = None


def _get_runner():
    """Cached jitted multi-core runner (mirrors bass2jax.run_bass_via_pjrt's
    multi-core path, but reuses the compiled executable across calls)."""
    global _RUNNER_CACHE
    if _RUNNER_CACHE is not None:
        return _RUNNER_CACHE

    import jax
    import jax.numpy as jnp
    from jax.experimental.shard_map import shard_map
    from jax.sharding import Mesh, PartitionSpec

    import concourse.mybir as mybir_
    from concourse import bass2jax

    nc = _get_nc()
    bass2jax.install_neuronx_cc_hook()

    in_names, out_names, out_avals, zero_outs = [], [], [], []
    for alloc in nc.m.functions[0].allocations:
        if not isinstance(mybir_.MemoryLocationSet, type) or not isinstance(
            alloc, mybir_.MemoryLocationSet
        ):
            continue
        name = alloc.memorylocations[0].name
        if alloc.kind == "ExternalInput":
            in_names.append(name)
        elif alloc.kind == "ExternalOutput":
            shape = tuple(alloc.tensor_shape)
            dtype = mybir_.dt.np(alloc.dtype)
            out_names.append(name)
            out_avals.append(jax.core.ShapedArray(shape, dtype))
            zero_outs.append(np.zeros(shape, dtype))
    n_params = len(in_names)
    n_outs = len(out_avals)
    all_in_names = list(in_names) + list(out_names)
    donate = tuple(range(n_params, n_params + n_outs))

    def _body(*args):
        outs = bass2jax._bass_exec_p.bind(
            *args,
            out_avals=tuple(out_avals),
            in_names=tuple(all_in_names),
            out_names=tuple(out_names),
            lowering_input_output_aliases=(),
            sim_require_finite=True,
            sim_require_nnan=True,
            nc=nc,
        )
        return tuple(outs)

    devices = jax.devices()[:NCORES]
    mesh = Mesh(np.asarray(devices), ("core",))
    sharded = jax.jit(
        shard_map(
            _body,
            mesh=mesh,
            in_specs=(PartitionSpec("core"),) * (n_params + n_outs),
            out_specs=(PartitionSpec("core"),) * n_outs,
            check_rep=False,
        ),
        donate_argnums=donate,
        keep_unused=True,
    )

    def runner(in_maps):
        concat_in = [
            np.concatenate([np.asarray(m[name]) for m in in_maps], axis=0)
            for name in in_names
        ]
        concat_zeros = [
            np.zeros((NCORES * z.shape[0], *z.shape[1:]), z.dtype) for z in zero_outs
        ]
        out_arrs = sharded(*concat_in, *concat_zeros)
        return [
            {
                name: np.asarray(out_arrs[i]).reshape(NCORES, *out_avals[i].shape)[c]
                for i, name in enumerate(out_names)
            }
            for c in range(NCORES)
        ]

    _RUNNER_CACHE = runner
    return runner


def run(inputs, trace=False):
    """Run the bass kernel on 8 cores. inputs = the reference input dict.
    Returns (full_output [16,256,32,32], BassKernelResults)."""
    x = np.ascontiguousarray(
        np.asarray(inputs["x"], dtype=np.float32)
        .astype(__import__("ml_dtypes").bfloat16)
    ).reshape(B, CH, L)
    consts = pack_inputs(
        np.asarray(inputs["w_qkv"], np.float32),
        np.asarray(inputs["s_qkv"], np.float32),
        np.asarray(inputs["b_qkv"], np.float32),
        np.asarray(inputs["w_dw"], np.float32),
        np.asarray(inputs["s_dw"], np.float32),
        np.asarray(inputs["b_dw"], np.float32),
        np.asarray(inputs["w_pw"], np.float32),
        np.asarray(inputs["s_pw"], np.float32),
        np.asarray(inputs["b_pw"], np.float32),
    )
    in_maps = []
    for c in range(NCORES):
        m = dict(consts)
        m["x"] = np.ascontiguousarray(x[c * BL : (c + 1) * BL])
        in_maps.append(m)

    nc = _get_nc()
    res = run_bass_kernel_spmd(
        nc, in_maps, core_ids=list(range(NCORES)), trace=trace
    )
    out = np.concatenate([r["out"] for r in res.results], axis=0)
    return out.reshape(B, CH, HH, WW), res


def kernel(**inputs) -> np.ndarray:
    out, _ = run(inputs, trace=False)
    return out



# revision 74
# speedup vs baseline: 1.0546x; 1.0055x over previous
"""Trainium2 Bass kernel for nn_Attention_50500225466997.

Computation (per batch): qkv = BN(conv1x1(x)); 4-head attention over L=1024
(DK=32, DH=64); out = attn + BN(dwconv3x3(v)); y = BN(conv1x1(out)).

Strategy:
  - Data-parallel over batch: 16 batches -> 8 NeuronCores, 2 per core.
  - All BN scales/permutations/SCALE folded into weights on the host.
  - Scores computed TRANSPOSED (S^T[l, m], l on partitions) so softmax
    needs no P-transposes: E = exp(S^T) unnormalized; Z rides FREE in the
    V-matmul by augmenting each head's vT lhsT with a 64-wide ones block
    ([vT_e|1] / [1|vT_o]) -> banks [O_e;Z_e] / [Z_o;O_o]; 1/Z via one
    exact DVE reciprocal, halves swapped by SBUF->SBUF DMA.
  - fp32r matmuls (fast fp32 streaming mode, ~tf32-accumulate-fp32).
  - Depthwise 3x3 via 9 diagonal-matrix matmuls accumulating in PSUM.
  - Cross-batch software pipelining: batch b+1's qkv/transposes emitted
    before batch b's depthwise/pointwise tail.
"""

import numpy as np

import concourse.bass as bass
import concourse.mybir as mybir
import concourse.tile as tile
from concourse import bacc
from concourse.bass_utils import run_bass_kernel_spmd

F32 = mybir.dt.float32
F32R = mybir.dt.float32r
AF = mybir.ActivationFunctionType
OP = mybir.AluOpType

B, CH, HH, WW = 16, 256, 32, 32
L = HH * WW                   # 1024
NH, DK, DH = 4, 32, 64
CQKV = CH + DK * NH * 2       # 512
SCALE = DK ** (-0.5)
NCORES = 8
BL = B // NCORES              # batches per core

# dtype config for precision experiments: flags switch a stage's matmul
# operands to full fp32 (4 cyc/row) instead of fp32r (1 cyc/row).
CFG = {"qkv32": False, "sc32": False, "o32": False, "pw32": False, "dw32": False, "o_bf16": True}




def build_bass():
    nc = bacc.Bacc("TRN2", target_bir_lowering=False, debug=False)
    BF16 = mybir.dt.bfloat16
    DT_XW = BF16
    DT_QK = BF16
    DT_VT = BF16 if CFG["o_bf16"] else (F32 if CFG["o32"] else F32R)
    DT_PW = BF16
    DT_DW = BF16

    x_d = nc.dram_tensor("x", [BL, CH, L], DT_XW, kind="ExternalInput")
    wqkvT_d = nc.dram_tensor("wqkvT", [128, 2, CQKV], DT_XW, kind="ExternalInput")
    bqkv_d = nc.dram_tensor("bqkv", [128, 4], F32, kind="ExternalInput")
    wpwT_d = nc.dram_tensor("wpwT", [128, 2, CH], DT_PW, kind="ExternalInput")
    bpw_d = nc.dram_tensor("bpw", [128, 2], F32, kind="ExternalInput")
    diag_d = nc.dram_tensor("diag", [128, 18, 128], DT_DW, kind="ExternalInput")
    id2_d = nc.dram_tensor("id2", [128, 64], BF16, kind="ExternalInput")
    ones_d = nc.dram_tensor("ones512", [128, 512], DT_VT, kind="ExternalInput")
    out_d = nc.dram_tensor("out", [BL, CH, L], F32, kind="ExternalOutput")

    with tile.TileContext(nc) as tc, nc.allow_low_precision(reason="fp32r"):
        with (
            tc.tile_pool(name="consts", bufs=1) as consts,
            tc.tile_pool(name="xin", bufs=4) as xin,
            tc.tile_pool(name="qkv", bufs=2) as qkvp,
            tc.tile_pool(name="vt", bufs=1) as vtp,
            tc.tile_pool(name="et", bufs=8) as etp,
            tc.tile_pool(name="o2", bufs=4) as o2p,
            tc.tile_pool(name="small", bufs=6) as smallp,
            tc.tile_pool(name="pad", bufs=4) as padp,
            tc.tile_pool(name="outp", bufs=4) as outp,
            tc.tile_pool(name="psc", bufs=2, space="PSUM") as psc,
            tc.tile_pool(name="pO", bufs=2, space="PSUM") as pOp,
            tc.tile_pool(name="pwork", bufs=2, space="PSUM") as pwork,
        ):
            # ---------------- constants (qkv path first) ----------------
            # DMA order tuned for startup latency: K/Q weights first, then
            # batch-0 x in half-tiles (first compute needs only the first
            # halves).  Startup DMAs are spread across issuing engines so
            # the per-DMA fixed overhead overlaps (separate HWDGE FIFOs).
            wqkvT = consts.tile([128, 2, CQKV], DT_XW)
            bqkv = consts.tile([128, 4], F32)
            nc.sync.dma_start(bqkv, bqkv_d.ap())
            nc.sync.dma_start(wqkvT[:, :, 0:256], wqkvT_d.ap()[:, :, 0:256])
            id2 = consts.tile([128, 64], BF16)
            ones512 = consts.tile([128, 8, 64], DT_VT)
            wpwT = consts.tile([128, 2, CH], DT_PW)
            bpw = consts.tile([128, 2], F32)
            diag = consts.tile([128, 18, 128], DT_DW)

            # prefetch all batches' x (xin bufs cover BL*2 tiles)
            Xall = []
            for b in range(BL):
                Xb = []
                for ct in range(2):
                    xt = xin.tile([128, L], DT_XW, name=f"x_b{b}c{ct}", tag="x")
                    Xb.append(xt)
                if b == 0:
                    # quarter-granularity for the first half (first qkv
                    # chunks start sooner), halves for the rest
                    qeng = [nc.scalar, nc.gpsimd]
                    for q in range(2):
                        for ct in range(2):
                            qs = slice(256 * q, 256 * q + 256)
                            qeng[ct].dma_start(
                                Xb[ct][:, qs],
                                x_d.ap()[b, 128 * ct : 128 * ct + 128, qs],
                            )
                    for ct in range(2):
                        hs = slice(512, 1024)
                        qeng[ct].dma_start(
                            Xb[ct][:, hs], x_d.ap()[b, 128 * ct : 128 * ct + 128, hs]
                        )
                    nc.gpsimd.dma_start(id2, id2_d.ap())
                    nc.gpsimd.dma_start(
                        wqkvT[:, :, 256:512], wqkvT_d.ap()[:, :, 256:512]
                    )
                    nc.sync.dma_start(
                        ones512, ones_d.ap().rearrange("p (c d) -> p c d", c=8)
                    )
                else:
                    for ct in range(2):
                        nc.sync.dma_start(
                            Xb[ct], x_d.ap()[b, 128 * ct : 128 * ct + 128, :]
                        )
                Xall.append(Xb)

            # persistent vt tiles: ones-halves written once
            Vt = []
            for h in range(NH):
                par = h % 2
                vt_h = vtp.tile([128, 8, 128], DT_VT, name=f"vt{h}", tag=f"vt{h}")
                nc.vector.tensor_copy(
                    vt_h[:, :, 64 - 64 * par : 128 - 64 * par], ones512
                )
                Vt.append(vt_h)

            # persistent padded-v images (bf16): border zeroed once; the qkv
            # V-evacuation writes straight into the interior, so no GpSimd
            # pad copies are needed.
            Pads = []
            for b in range(BL):
                pb = []
                for ct in range(2):
                    padt = padp.tile(
                        [128, 34, 34], DT_DW, name=f"pad{b}{ct}", tag=f"pad{b}{ct}"
                    )
                    nc.vector.memset(padt, 0.0)
                    pb.append(padt)
                Pads.append(pb)

            # ---- staged emission with cross-batch software pipelining ----
            st = [{} for _ in range(BL)]

            def emit_head(b):
                X = Xall[b]
                Qa = qkvp.tile([128, L], DT_QK, name=f"Qa_{b}", tag="Qa")
                Ka = qkvp.tile([128, L], DT_QK, name=f"Ka_{b}", tag="Ka")
                Vf = [
                    qkvp.tile([128, L], DT_VT, name=f"Vf_{b}{ct}", tag=f"Vf{ct}")
                    for ct in range(2)
                ]
                for mt in range(2):  # mt outer: consume x halves as they land
                    cw = 256 if (b == 0 and mt == 0) else 512
                    for ot in (1, 0, 2, 3):
                        for off in range(512 * mt, 512 * mt + 512, cw):
                            ms = slice(off, off + cw)
                            pq = pwork.tile(
                                [128, cw], F32, name=f"pq{b}{ot}{off}", tag="w"
                            )
                            for kt in range(2):
                                nc.tensor.matmul(
                                    pq,
                                    wqkvT[:, kt, 128 * ot : 128 * ot + 128],
                                    X[kt][:, ms],
                                    start=(kt == 0),
                                    stop=(kt == 1),
                                )
                            if ot < 2:
                                nc.vector.tensor_scalar(
                                    [Qa, Ka][ot][:, ms], pq, bqkv[:, ot : ot + 1],
                                    None, OP.add,
                                )
                            else:
                                # V: evacuate twice — padded image interior
                                # (DVE, depthwise) and flat (ACT, transposes).
                                nr = cw // 32
                                nc.vector.tensor_scalar(
                                    Pads[b][ot - 2][
                                        :, 1 + off // 32 : 1 + off // 32 + nr, 1:33
                                    ],
                                    pq.rearrange("p (a c) -> p a c", a=nr),
                                    bqkv[:, ot : ot + 1],
                                    None, OP.add,
                                )
                                nc.scalar.activation(
                                    Vf[ot - 2][:, ms], pq, AF.Identity,
                                    bias=bqkv[:, ot : ot + 1],
                                )
                if b == 0:
                    # late consts: needed only by dw/pw phases
                    nc.scalar.dma_start(wpwT, wpwT_d.ap())
                    nc.gpsimd.dma_start(bpw, bpw_d.ap())
                    nc.gpsimd.dma_start(diag, diag_d.ap())

                # vT transposes into the persistent augmented tiles
                for h in range(NH):
                    ct, lo = h // 2, (h % 2) * 64
                    pv = pwork.tile([128, 512], DT_VT, name=f"pv{b}{h}", tag="w")
                    for c8 in range(8):
                        nc.tensor.transpose(
                            pv[:, 64 * c8 : 64 * c8 + 64],
                            Vf[ct][lo : lo + 64, 128 * c8 : 128 * c8 + 128],
                            id2[lo : lo + 64, :],
                        )
                    par = h % 2
                    nc.vector.tensor_copy(
                        Vt[h][:, :, 64 * par : 64 * par + 64],
                        pv.rearrange("p (c d) -> p c d", c=8),
                    )

                out2 = [
                    o2p.tile([128, L], DT_PW, name=f"o2_{b}{ct}", tag="o2")
                    for ct in range(2)
                ]
                st[b].update(Qa=Qa, Ka=Ka, pads=Pads[b], out2=out2)

            ID_MASK = list(range(32))

            def emit_attn(b, hp, mts=(0, 1), fine=False):
                # per (hp, mt): bank A = [vT_e|1].T @ E_e = [O_e; Z_e],
                # bank B = [1|vT_o].T @ E_o = [Z_o; O_o]; 1/Z via one fast
                # approx reciprocal; halves swapped with a DVE shuffle.
                Qa, Ka, out2 = st[b]["Qa"], st[b]["Ka"], st[b]["out2"]
                for mt in mts:
                    ms = slice(512 * mt, 512 * mt + 512)
                    pA = pOp.tile([128, 512], F32, name=f"pa{b}{hp}{mt}", tag="o")
                    pB = pOp.tile([128, 512], F32, name=f"pb{b}{hp}{mt}", tag="o")
                    banks = [pA, pB]
                    for lt in range(8):
                        ls = slice(128 * lt, 128 * lt + 128)
                        sc = psc.tile(
                            [128, 1024], F32, name=f"sc{b}{hp}{mt}{lt}", tag="sc"
                        )
                        for j in range(2):
                            h = 2 * hp + j
                            nc.tensor.matmul(
                                sc[:, 512 * j : 512 * j + 512],
                                Ka[32 * h : 32 * h + 32, ls],
                                Qa[32 * h : 32 * h + 32, ms],
                                start=True,
                                stop=True,
                                tile_position=(32 * h, 0),
                            )
                        Et = etp.tile(
                            [128, 1024], DT_VT, name=f"e{b}{hp}{mt}{lt}", tag="e"
                        )
                        nc.scalar.activation(Et, sc, AF.Exp)
                        for j in range(2):
                            h = 2 * hp + j
                            nc.tensor.matmul(
                                banks[j],
                                Vt[h][:, lt, :],
                                Et[:, 512 * j : 512 * j + 512],
                                start=(lt == 0),
                                stop=(lt == 7),
                                skip_group_check=True,
                            )
                    # stage Z (cross-half via shuffle-copy) and O halves to
                    # SBUF: ZA ends up swap-aligned with Ou, so no extra
                    # swap op is needed before the reciprocal.  `fine`
                    # chunks the chain to shorten tail latency.
                    steps = ((0, 256), (256, 256)) if fine else ((0, 512),)
                    for off, cw in steps:
                        cs = slice(off, off + cw)
                        os_ = slice(512 * mt + off, 512 * mt + off + cw)
                        ZA = smallp.tile(
                            [128, cw], F32, name=f"za{b}{hp}{mt}{off}", tag="za"
                        )
                        nc.vector.stream_shuffle(ZA[0:64, :], pA[64:128, cs], ID_MASK)
                        nc.vector.stream_shuffle(ZA[64:128, :], pB[0:64, cs], ID_MASK)
                        Ou = smallp.tile(
                            [128, cw], F32, name=f"ou{b}{hp}{mt}{off}", tag="ou"
                        )
                        nc.vector.tensor_copy(Ou[0:64, :], pA[0:64, cs])
                        nc.vector.tensor_copy(Ou[64:128, :], pB[64:128, cs])
                        Rz = smallp.tile(
                            [128, cw], F32, name=f"rz{b}{hp}{mt}{off}", tag="rz"
                        )
                        nc.vector.reciprocal_approx_fast(out=Rz, in_=ZA)
                        nc.vector.scalar_tensor_tensor(
                            out=out2[hp][:, os_],
                            in0=Ou,
                            scalar=1.0,
                            in1=Rz,
                            op0=OP.mult,
                            op1=OP.mult,
                        )

            def emit_dw(b, ct, mts=(0, 1), fine=False):
                padt, out2 = st[b]["pads"][ct], st[b]["out2"]
                for mt in mts:
                    dwp = pwork.tile([128, 512], F32, name=f"dw{b}{ct}{mt}", tag="w")
                    for tap in range(9):
                        dy, dx = tap // 3, tap % 3
                        r0 = 16 * mt + dy
                        nc.tensor.matmul(
                            dwp,
                            diag[:, 9 * ct + tap, :],
                            padt[:, r0 : r0 + 16, dx : dx + 32],
                            start=(tap == 0),
                            stop=(tap == 8),
                        )
                    steps = ((0, 256), (256, 256)) if fine else ((0, 512),)
                    for off, cw in steps:
                        os_ = slice(512 * mt + off, 512 * mt + off + cw)
                        nc.vector.scalar_tensor_tensor(
                            out=out2[ct][:, os_],
                            in0=dwp[:, off : off + cw],
                            scalar=1.0,
                            in1=out2[ct][:, os_],
                            op0=OP.mult,
                            op1=OP.add,
                        )

            def emit_pw(b, mt, fine=False):
                out2 = st[b]["out2"]
                steps = ((0, 256), (256, 256)) if fine else ((0, 512),)
                for off, cw in steps:
                    ms = slice(512 * mt + off, 512 * mt + off + cw)
                    for ot in range(2):
                        pp = pwork.tile(
                            [128, cw], F32, name=f"pp{b}{mt}{ot}{off}", tag="w"
                        )
                        for kt in range(2):
                            nc.tensor.matmul(
                                pp,
                                wpwT[:, kt, 128 * ot : 128 * ot + 128],
                                out2[kt][:, ms],
                                start=(kt == 0),
                                stop=(kt == 1),
                            )
                        osb = outp.tile(
                            [128, cw], F32, name=f"os{b}{mt}{ot}{off}", tag="os"
                        )
                        nc.vector.tensor_scalar(
                            osb, pp, bpw[:, ot : ot + 1], None, OP.add
                        )
                        nc.sync.dma_start(
                            out_d.ap()[b, 128 * ot : 128 * ot + 128, ms], osb
                        )

            assert BL == 2
            emit_head(0)
            emit_attn(0, 0)
            emit_dw(0, 0)
            emit_attn(0, 1)
            emit_head(1)          # batch-1 qkv before batch-0 tail work
            emit_dw(0, 1)
            emit_pw(0, 0)
            emit_pw(0, 1)
            emit_attn(1, 0)
            emit_dw(1, 0)
            emit_attn(1, 1, mts=(0,))
            emit_dw(1, 1, mts=(0,))
            emit_pw(1, 0)
            emit_attn(1, 1, mts=(1,), fine=True)
            emit_dw(1, 1, mts=(1,), fine=True)
            emit_pw(1, 1, fine=True)

    nc.compile()
    return nc


def pack_inputs(w_qkv, s_qkv, b_qkv, w_dw, s_dw, b_dw, w_pw, s_pw, b_pw):
    """Host-side weight packing. Returns dict of constant arrays (shared by
    all cores)."""
    f32 = np.float32
    Wq = (w_qkv[:, :, 0, 0] * s_qkv[:, None]).astype(np.float64)  # [512, 256]
    bq = b_qkv.astype(np.float64).copy()

    # permute output channels to [Q_all, K_all, V0, V1]
    perm = []
    for h in range(NH):
        perm += [h * 128 + d for d in range(32)]           # q
    for h in range(NH):
        perm += [h * 128 + 32 + d for d in range(32)]      # k
    for h in range(NH):
        perm += [h * 128 + 64 + d for d in range(64)]      # v
    perm = np.array(perm)
    Wq = Wq[perm]
    bq = bq[perm]
    # fold attention scale into q
    Wq[0:128] *= SCALE
    bq[0:128] *= SCALE

    wqkvT = np.ascontiguousarray(
        Wq.T.reshape(2, 128, CQKV).transpose(1, 0, 2)
    ).astype(__import__("ml_dtypes").bfloat16)  # [128, 2, 512]
    bqkv = np.ascontiguousarray(bq.reshape(4, 128).T).astype(f32)  # [128, 4]

    bf16_ = __import__("ml_dtypes").bfloat16
    Wp = (w_pw[:, :, 0, 0] * s_pw[:, None]).astype(np.float64)     # [256, 256]
    bp = b_pw.astype(np.float64) + Wp @ b_dw.astype(np.float64)
    wpwT = np.ascontiguousarray(
        Wp.T.reshape(2, 128, CH).transpose(1, 0, 2)
    ).astype(bf16_)  # [128, 2, 256]
    bpw = np.ascontiguousarray(bp.reshape(2, 128).T).astype(f32)   # [128, 2]

    bf16 = __import__("ml_dtypes").bfloat16
    wd = (w_dw[:, 0] * s_dw[:, None, None]).astype(f32)            # [256, 3, 3]
    diag = np.zeros((128, 18, 128), f32)
    for ct in range(2):
        for tap in range(9):
            dy, dx = tap // 3, tap % 3
            idx = np.arange(128)
            diag[idx, 9 * ct + tap, idx] = wd[128 * ct + idx, dy, dx]

    id2 = np.tile(np.eye(64, dtype=f32), (2, 1))                   # [128, 64]

    return {
        "wqkvT": wqkvT,
        "bqkv": bqkv,
        "wpwT": wpwT,
        "bpw": bpw,
        "diag": diag.astype(bf16),
        "id2": id2.astype(bf16),
        "ones512": np.ones((128, 512), bf16 if CFG["o_bf16"] else f32),
    }


_NC_CACHE = None


def _get_nc():
    global _NC_CACHE
    if _NC_CACHE is None:
        _NC_CACHE = build_bass()
    return _NC_CACHE


_RUNNER_CACHE = None


def _get_runner():
    """Cached jitted multi-core runner (mirrors bass2jax.run_bass_via_pjrt's
    multi-core path, but reuses the compiled executable across calls)."""
    global _RUNNER_CACHE
    if _RUNNER_CACHE is not None:
        return _RUNNER_CACHE

    import jax
    import jax.numpy as jnp
    from jax.experimental.shard_map import shard_map
    from jax.sharding import Mesh, PartitionSpec

    import concourse.mybir as mybir_
    from concourse import bass2jax

    nc = _get_nc()
    bass2jax.install_neuronx_cc_hook()

    in_names, out_names, out_avals, zero_outs = [], [], [], []
    for alloc in nc.m.functions[0].allocations:
        if not isinstance(mybir_.MemoryLocationSet, type) or not isinstance(
            alloc, mybir_.MemoryLocationSet
        ):
            continue
        name = alloc.memorylocations[0].name
        if alloc.kind == "ExternalInput":
            in_names.append(name)
        elif alloc.kind == "ExternalOutput":
            shape = tuple(alloc.tensor_shape)
            dtype = mybir_.dt.np(alloc.dtype)
            out_names.append(name)
            out_avals.append(jax.core.ShapedArray(shape, dtype))
            zero_outs.append(np.zeros(shape, dtype))
    n_params = len(in_names)
    n_outs = len(out_avals)
    all_in_names = list(in_names) + list(out_names)
    donate = tuple(range(n_params, n_params + n_outs))

    def _body(*args):
        outs = bass2jax._bass_exec_p.bind(
            *args,
            out_avals=tuple(out_avals),
            in_names=tuple(all_in_names),
            out_names=tuple(out_names),
            lowering_input_output_aliases=(),
            sim_require_finite=True,
            sim_require_nnan=True,
            nc=nc,
        )
        return tuple(outs)

    devices = jax.devices()[:NCORES]
    mesh = Mesh(np.asarray(devices), ("core",))
    sharded = jax.jit(
        shard_map(
            _body,
            mesh=mesh,
            in_specs=(PartitionSpec("core"),) * (n_params + n_outs),
            out_specs=(PartitionSpec("core"),) * n_outs,
            check_rep=False,
        ),
        donate_argnums=donate,
        keep_unused=True,
    )

    def runner(in_maps):
        concat_in = [
            np.concatenate([np.asarray(m[name]) for m in in_maps], axis=0)
            for name in in_names
        ]
        concat_zeros = [
            np.zeros((NCORES * z.shape[0], *z.shape[1:]), z.dtype) for z in zero_outs
        ]
        out_arrs = sharded(*concat_in, *concat_zeros)
        return [
            {
                name: np.asarray(out_arrs[i]).reshape(NCORES, *out_avals[i].shape)[c]
                for i, name in enumerate(out_names)
            }
            for c in range(NCORES)
        ]

    _RUNNER_CACHE = runner
    return runner


def run(inputs, trace=False):
    """Run the bass kernel on 8 cores. inputs = the reference input dict.
    Returns (full_output [16,256,32,32], BassKernelResults)."""
    x = np.ascontiguousarray(
        np.asarray(inputs["x"], dtype=np.float32)
        .astype(__import__("ml_dtypes").bfloat16)
    ).reshape(B, CH, L)
    consts = pack_inputs(
        np.asarray(inputs["w_qkv"], np.float32),
        np.asarray(inputs["s_qkv"], np.float32),
        np.asarray(inputs["b_qkv"], np.float32),
        np.asarray(inputs["w_dw"], np.float32),
        np.asarray(inputs["s_dw"], np.float32),
        np.asarray(inputs["b_dw"], np.float32),
        np.asarray(inputs["w_pw"], np.float32),
        np.asarray(inputs["s_pw"], np.float32),
        np.asarray(inputs["b_pw"], np.float32),
    )
    in_maps = []
    for c in range(NCORES):
        m = dict(consts)
        m["x"] = np.ascontiguousarray(x[c * BL : (c + 1) * BL])
        in_maps.append(m)

    nc = _get_nc()
    res = run_bass_kernel_spmd(
        nc, in_maps, core_ids=list(range(NCORES)), trace=trace
    )
    out = np.concatenate([r["out"] for r in res.results], axis=0)
    return out.reshape(B, CH, HH, WW), res


def kernel(**inputs) -> np.ndarray:
    out, _ = run(inputs, trace=False)
    return out

